# revision 5
# baseline (speedup 1.0000x reference)
"""Block-local attention (BlockLocalAttentionProduct) Trainium2 Bass kernel.

Strategy (per NeuronCore; (n*h)=24 heads split 3-per-core over 8 cores):
  - Query "chunks" of 128 rows (= 2 reference blocks of 64). Chunk c
    attends padded key tiles c and c+1 (128 keys each) + 64 globals.
  - scoresT layout: K-tile @ Q^T -> (k on partitions, q free), exp on
    ScalarE (scale=1/8, no max-subtraction), block-structure masking via
    two 64x64 rect zero-memsets per key-tile pair. Pad + user attention
    masks are folded into V host-side: exp(s+m) = e^m * e^s and
    ctx = (diag(e^m) [V|1])^T @ exp(S), so the V' ones-column carries
    both the mask and the softmax denominator.
  - PV: stationary V'-tile (128 x 65), moving exp(scoresT) pairs
    (N=256), accumulated into a global ctxT column space in PSUM using
    per-element has_written accumulate semantics (overlapping outputs).
  - Global tokens: G^T @ Q^T scores in 512-wide groups, exp'd to an
    es2 buffer, added into the same ctxT PSUM banks via G-V' matmuls on
    even pairs.
  - Tail per 4-chunk group: ctxT bank -> SBUF, per-chunk PE transpose
    back to natural (128 x 65) incl. denominator column, reciprocal +
    per-chunk scalar multiply -> output staging -> batched DMA out.
  - All matmul operands in f32r (fp32 w/ 11-bit mantissa, full-rate for
    moving dim >= 256); inputs pre-rounded to the f32r grid on host.
"""

import numpy as np

import concourse.bass as bass  # noqa: F401
import concourse.mybir as mybir
import concourse.tile as tile
from concourse import bacc
from concourse.bass_utils import run_bass_kernel_spmd
from concourse.masks import make_identity

F32 = mybir.dt.float32
F32R = mybir.dt.float32r
AF = mybir.ActivationFunctionType

D = 64          # head dim
QC = 128        # queries per chunk
KTILE = 128     # keys per tile
G = 64          # global tokens
N_CORES = 8


def _emit_head(nc, tc, pools, i, nh, t):
    (constp, pernh, stage, ktsp, pbp, tailp, outsp,
     ps_s, ps_tp, ps_pc, ps_cn, identf, identr) = pools
    nch = t // QC
    ntile = nch + 1
    ngrp = nch // 4

    dq, dk, dv, dgk, dgv, dout = (nc._dr[k] for k in
                                  ("q", "kp", "vpe", "gk", "gve", "out"))

    # ---- per-head persistent buffers ----
    qtb = pernh.tile([64, t], F32R, tag="qtb")       # Q^T (d x t)
    es2 = pernh.tile([64, t], F32R, tag="es2")       # exp(global scoresT)
    gts = pernh.tile([64, G], F32R, tag="gts")       # G^T (d x g)
    gvs = pernh.tile([64, G + 1], F32R, tag="gvs")   # global [V|1] * e^gm

    # ---- global K/V prep ----
    gn = stage.tile([64, D], F32R, tag="gn")
    nc.sync.dma_start(gn, dgk[i, :, :])
    nc.sync.dma_start(gvs, dgv[i, :, :])
    tpg = ps_tp.tile([64, 512], F32R, tag="tpq")
    nc.tensor.transpose(tpg[:, 0:G], gn, identr[0:64, 0:64])
    nc.vector.tensor_copy(gts, tpg[:, 0:G])

    # ========== Pass 1: staging + transposes + global scores ==========
    kts_by_group = {}
    ve_by_batch = {}
    qn = kn = tq = tk = None
    for c in range(nch):
        if c % 8 == 0:
            b = c // 8
            qn = stage.tile([128, 8, D], F32R, tag="qn")
            nc.sync.dma_start(
                qn, dq[i, c * QC:(c + 8) * QC, :].rearrange(
                    "(b p) d -> p b d", p=128))
            kn = stage.tile([128, 8, D], F32R, tag="kn")
            nc.sync.dma_start(
                kn, dk[i, c * KTILE:(c + 8) * KTILE, :].rearrange(
                    "(b p) d -> p b d", p=128))
            ve = stage.tile([128, 8, D + 1], F32R, tag="ve")
            nc.sync.dma_start(
                ve, dv[i, c * KTILE:(c + 8) * KTILE, :].rearrange(
                    "(b p) d -> p b d", p=128))
            ve_by_batch[b] = ve
        if c % 4 == 0:
            tq = ps_tp.tile([64, 512], F32R, tag="tpq")
            tk = ps_tp.tile([64, 512], F32R, tag="tpk")
        co = (c % 4) * 128
        nc.tensor.transpose(tq[:, co:co + 128], qn[:, c % 8, :], identr)
        nc.tensor.transpose(tk[:, co:co + 128], kn[:, c % 8, :], identr)
        if c % 4 == 3:
            g = c // 4
            nc.vector.tensor_copy(qtb[:, g * 512:(g + 1) * 512], tq)
            kts = ktsp.tile([64, 512], F32R, tag="kts")
            nc.vector.tensor_copy(kts, tk)
            kts_by_group[g] = kts
            # global scores for this 4-chunk group
            s2 = ps_tp.tile([64, 512], F32, tag="s2")
            nc.tensor.matmul(s2, gts, qtb[:, g * 512:(g + 1) * 512],
                             start=True, stop=True)
            nc.scalar.activation(es2[:, g * 512:(g + 1) * 512], s2,
                                 AF.Exp, scale=0.125)

    # final half-pad key tile (index nch)
    kn65 = stage.tile([128, D], F32R, tag="kn65")
    nc.sync.dma_start(kn65, dk[i, nch * KTILE:(nch + 1) * KTILE, :])
    ve65 = stage.tile([128, D + 1], F32R, tag="ve65")
    nc.sync.dma_start(ve65, dv[i, nch * KTILE:(nch + 1) * KTILE, :])
    tk65 = ps_tp.tile([64, 512], F32R, tag="tpk")
    nc.tensor.transpose(tk65[:, 0:128], kn65, identr)
    kts65 = ktsp.tile([64, 512], F32R, tag="kts")
    nc.vector.tensor_copy(kts65[:, 0:128], tk65[:, 0:128])

    # ========== Pass 2+3+4 interleaved over key-tile pairs ==========
    sbatch = None
    pcx = {}         # ctxT bank index -> psum tensor (65, 512)
    outs_by_ob = {}

    def bank(gidx):
        bt = pcx.get(gidx)
        if bt is None:
            bt = ps_pc.tile([65, 512], F32, tag="pcx")
            pcx[gidx] = bt
        return bt

    def do_pv_tail(jj, pb):
        # ---- PV pair jj into global ctxT col space ----
        vsl = (ve_by_batch[jj // 8][:, jj % 8, :] if jj < nch else ve65)
        soff_j = (jj % 2) * 256
        pieces = []
        if jj > 0:
            pieces.append((jj - 1, soff_j))
        if jj < nch:
            pieces.append((jj, soff_j + (128 if jj > 0 else 0)))
        if len(pieces) == 2 and (jj - 1) // 4 == jj // 4:
            bt = bank(jj // 4)
            rel = ((jj - 1) % 4) * 128
            nc.tensor.matmul(bt[:, rel:rel + 256], vsl,
                             pb[:, soff_j:soff_j + 256],
                             start=False, stop=False, skip_group_check=True)
        else:
            for (cc, po) in pieces:
                bt = bank(cc // 4)
                rel = (cc % 4) * 128
                nc.tensor.matmul(bt[:, rel:rel + 128], vsl,
                                 pb[:, po:po + 128],
                                 start=(cc % 4 == 0), stop=False,
                                 skip_group_check=True)

        # ---- PVg on even pairs (each chunk's global add lands once) ----
        if jj % 2 == 0:
            gpieces = []
            if jj > 0:
                gpieces.append((jj - 1, (jj - 1) * QC))
            if jj < nch:
                gpieces.append((jj, jj * QC))
            if len(gpieces) == 2 and (jj - 1) // 4 == jj // 4:
                bt = bank(jj // 4)
                rel = ((jj - 1) % 4) * 128
                nc.tensor.matmul(bt[:, rel:rel + 256], gvs,
                                 es2[:, (jj - 1) * QC:(jj + 1) * QC],
                                 start=False, stop=False,
                                 skip_group_check=True)
            else:
                for (cc, eo) in gpieces:
                    bt = bank(cc // 4)
                    rel = (cc % 4) * 128
                    nc.tensor.matmul(bt[:, rel:rel + 128], gvs,
                                     es2[:, eo:eo + 128],
                                     start=False, stop=False,
                                     skip_group_check=True)

        # ---- tail for any completed bank ----
        done_g = None
        if jj % 4 == 0 and jj >= 4:
            done_g = jj // 4 - 1
        if jj == ntile - 1:
            done_g = ngrp - 1
        if done_g is not None and done_g in pcx:
            g = done_g
            bt = pcx.pop(g)
            ctxs = tailp.tile([65, 512], F32, tag="ctxs")
            nc.scalar.activation(ctxs, bt, AF.Copy)
            ctxn = ps_cn.tile([128, 4, 65], F32, tag="ctxn")
            for cc in range(4):
                nc.tensor.transpose(ctxn[:, cc, :],
                                    ctxs[:, cc * 128:(cc + 1) * 128],
                                    identf[0:65, 0:65])
            rec4 = tailp.tile([128, 4, 1], F32, tag="rec4")
            nc.vector.reciprocal(rec4, ctxn[:, :, 64:65])
            ob = g // 2
            if ob not in outs_by_ob:
                outs_by_ob[ob] = outsp.tile([128, 8, D], F32, tag="outs", name="outs")
            outs = outs_by_ob[ob]
            for cc in range(4):
                nc.vector.tensor_scalar_mul(
                    outs[:, (g % 2) * 4 + cc, :], ctxn[:, cc, 0:64],
                    rec4[:, cc, :])
            if g % 2 == 1:
                del outs_by_ob[ob]
                nc.sync.dma_start(
                    dout[i, ob * 8 * QC:(ob + 1) * 8 * QC, :].rearrange(
                        "(b p) d -> p b d", p=128),
                    outs)

    for j in range(ntile):
        # ---- QK pair j ----
        q0 = max(0, j - 1) * QC
        q1 = min(nch, j + 1) * QC
        width = q1 - q0
        soff = (j % 2) * 256
        if j % 2 == 0:
            sbatch = ps_s.tile([128, 512], F32, tag="s")
        if width < 256:
            # zero unwritten psum so exp never sees stale junk
            nc.vector.memset(sbatch[:, soff + width:soff + 256], 0.0)
            if j == ntile - 1 and j % 2 == 0:
                nc.vector.memset(sbatch[:, 256:512], 0.0)
        if j < nch:
            ktg = kts_by_group[j // 4]
            ksl = ktg[:, (j % 4) * 128:(j % 4) * 128 + 128]
        else:
            ksl = kts65[:, 0:128]
        nc.tensor.matmul(sbatch[:, soff:soff + width], ksl, qtb[:, q0:q1],
                         start=True, stop=True)

        # ---- exp + rect masks + PV/tails when batch complete ----
        if j % 2 == 1 or j == ntile - 1:
            pb = pbp.tile([128, 512], F32R, tag="pb")
            nc.scalar.activation(pb, sbatch, AF.Exp, scale=0.125)
            j0 = j - (j % 2)
            for jj in (j0, j0 + 1):
                if jj >= ntile:
                    continue
                base = (jj % 2) * 256
                if jj > 0:
                    # chunk jj-1 at cols base:base+128; q-half0 can't see
                    # k rows [64:128) of tile jj
                    nc.vector.memset(
                        pb[64:128, base:base + 64].bitcast(F32), 0.0)
                if jj < nch:
                    # chunk jj; q-half1 can't see k rows [0:64) of tile jj
                    cb = base if jj == 0 else base + 128
                    nc.vector.memset(
                        pb[0:64, cb + 64:cb + 128].bitcast(F32), 0.0)
            for jj in (j0, j0 + 1):
                if jj < ntile:
                    do_pv_tail(jj, pb)


def build_kernel(nh: int, t: int):
    """Build the Bass program for one core processing `nh` heads of length `t`."""
    assert (t // QC) % 8 == 0
    nc = bacc.Bacc(None, target_bir_lowering=False)

    nc._dr = {
        "q": nc.dram_tensor("q", [nh, t, D], F32R, kind="ExternalInput"),
        "kp": nc.dram_tensor("kp", [nh, t + KTILE, D], F32R,
                             kind="ExternalInput"),
        "vpe": nc.dram_tensor("vpe", [nh, t + KTILE, D + 1], F32R,
                              kind="ExternalInput"),
        "gk": nc.dram_tensor("gk", [nh, G, D], F32R, kind="ExternalInput"),
        "gve": nc.dram_tensor("gve", [nh, G, D + 1], F32R,
                              kind="ExternalInput"),
        "out": nc.dram_tensor("out", [nh, t, D], F32, kind="ExternalOutput"),
    }

    with tile.TileContext(nc) as tc:
        with (
            tc.tile_pool(name="const", bufs=1) as constp,
            tc.tile_pool(name="pernh", bufs=1) as pernh,
            tc.tile_pool(name="stage", bufs=3) as stage,
            tc.tile_pool(name="kts", bufs=3) as ktsp,
            tc.tile_pool(name="pb", bufs=3) as pbp,
            tc.tile_pool(name="tail", bufs=2) as tailp,
            tc.tile_pool(name="outs", bufs=2) as outsp,
            tc.tile_pool(name="ps_s", bufs=2, space="PSUM") as ps_s,
            tc.tile_pool(name="ps_tp", bufs=1, space="PSUM") as ps_tp,
            tc.tile_pool(name="ps_pc", bufs=2, space="PSUM") as ps_pc,
            tc.tile_pool(name="ps_cn", bufs=1, space="PSUM") as ps_cn,
        ):
            identf = constp.tile([128, 128], F32)
            make_identity(nc, identf)
            identr = constp.tile([128, 128], F32R)
            nc.vector.tensor_copy(identr, identf)
            pools = (constp, pernh, stage, ktsp, pbp, tailp, outsp,
                     ps_s, ps_tp, ps_pc, ps_cn, identf, identr)
            for i in range(nh):
                _emit_head(nc, tc, pools, i, nh, t)

    nc.finalize()
    return nc


# ----------------------------------------------------------------------------
# host-side wrapper
# ----------------------------------------------------------------------------

_CACHE = {}


def _round_f32r(a):
    b = np.ascontiguousarray(a, dtype=np.float32).view(np.uint32)
    b = (b + np.uint32(0x800)) & np.uint32(0xFFFFF000)
    return b.view(np.float32)


def prep_inputs(query_layer, key_layer, value_layer, attention_mask,
                global_key, global_value, global_mask, n_cores=N_CORES):
    n, h, t, d = query_layer.shape
    g = global_key.shape[-2]
    assert d == D and g == G
    nheads = n * h
    assert nheads % n_cores == 0
    nh = nheads // n_cores

    q = _round_f32r(query_layer.reshape(nheads, t, d))

    half = KTILE // 2
    kp = np.zeros((nheads, t + KTILE, d), np.float32)
    kp[:, half:half + t, :] = key_layer.reshape(nheads, t, d)
    kp = _round_f32r(kp)

    # additive masks -> multiplicative fold into V' (incl. pads)
    mpad = np.full((n, t + KTILE), -10000.0, np.float64)
    mpad[:, half:half + t] = attention_mask.reshape(n, t).astype(np.float64)
    em = np.exp(mpad).astype(np.float32)                       # (n, t+KT)
    vp = np.zeros((nheads, t + KTILE, d + 1), np.float32)
    vp[:, half:half + t, :d] = value_layer.reshape(nheads, t, d)
    vp[:, :, d] = 1.0
    emh = np.repeat(em, h, axis=0)
    vpe = _round_f32r(vp * emh[:, :, None])

    gk = _round_f32r(global_key.reshape(nheads, g, d))
    egm = np.exp(global_mask.reshape(n, g).astype(np.float64)).astype(np.float32)
    gvf = np.concatenate(
        [global_value.reshape(nheads, g, d),
         np.ones((nheads, g, 1), np.float32)], axis=2)
    gve = _round_f32r(gvf * np.repeat(egm, h, axis=0)[:, :, None])

    in_maps = []
    for c in range(n_cores):
        s = slice(c * nh, (c + 1) * nh)
        in_maps.append({
            "q": np.ascontiguousarray(q[s]),
            "kp": np.ascontiguousarray(kp[s]),
            "vpe": np.ascontiguousarray(vpe[s]),
            "gk": np.ascontiguousarray(gk[s]),
            "gve": np.ascontiguousarray(gve[s]),
        })
    return in_maps, nh


def kernel(query_layer, key_layer, value_layer, attention_mask,
           global_key, global_value, global_mask):
    query_layer = np.asarray(query_layer, dtype=np.float32)
    key_layer = np.asarray(key_layer, dtype=np.float32)
    value_layer = np.asarray(value_layer, dtype=np.float32)
    attention_mask = np.asarray(attention_mask, dtype=np.float32)
    global_key = np.asarray(global_key, dtype=np.float32)
    global_value = np.asarray(global_value, dtype=np.float32)
    global_mask = np.asarray(global_mask, dtype=np.float32)

    n, h, t, d = query_layer.shape
    in_maps, nh = prep_inputs(query_layer, key_layer, value_layer,
                              attention_mask, global_key, global_value,
                              global_mask)
    key = (nh, t)
    if key not in _CACHE:
        _CACHE[key] = build_kernel(nh, t)
    nc = _CACHE[key]

    res = run_bass_kernel_spmd(nc, in_maps, core_ids=list(range(N_CORES)))
    out = np.concatenate([r["out"] for r in res.results], axis=0)
    return out.reshape(n, h, t, d)
